# revision 11
# baseline (speedup 1.0000x reference)
"""Mamba block (MockMambaBlock) on 8 Trainium2 NeuronCores.

Sharding: tensor-parallel over d_inner (8 x 256 channels), both batches on
every core. The x_proj/dt_proj contraction over d_inner is completed with an
on-device AllReduce of the small (B, 32, L) partial; out_proj row-partials
are summed on the host (the gather step).

v2: depthwise conv moved to the PE (per-tap diagonal matmuls), dtx/gating
moved to the Pool engine, and phase B of batch 0 overlaps phase A of batch 1
(interleaved emission) so the DVE scan chain never starves.
"""

import sys

sys.path.insert(0, "/opt/trn_rl_repo")

import numpy as np
import ml_dtypes

import concourse.bass as bass
import concourse.bacc as bacc
import concourse.mybir as mybir
import concourse.tile as tile
from concourse.bass_utils import run_bass_kernel_spmd

F32 = mybir.dt.float32
BF16 = mybir.dt.bfloat16
AF = mybir.ActivationFunctionType
OP = mybir.AluOpType

B, L, DM, DI, DS, DC = 2, 2048, 1024, 2048, 16, 4
NCORES = 8
DIL = DI // NCORES          # 256 channels per core
NBLK = DIL // 128           # 2 partition blocks of channels
KBLK = DM // 128            # 8 contraction blocks for in_proj
LTA = 512                   # phase A token chunk
NPT = L // 512              # 512-token psum tiles in phase B


def build_nc():
    nc = bacc.Bacc()

    x_t = nc.dram_tensor("x_t", [B, KBLK, 128, L], BF16, kind="ExternalInput")
    win_d = nc.dram_tensor("win", [DM, 2 * DIL], BF16, kind="ExternalInput")
    wout_d = nc.dram_tensor("wout", [DIL, DM], BF16, kind="ExternalInput")
    wx_d = nc.dram_tensor("wx", [DIL, 2 * DS], BF16, kind="ExternalInput")
    wdt_d = nc.dram_tensor("wdt", [DS, DIL], BF16, kind="ExternalInput")
    a_d = nc.dram_tensor("a", [DIL, DS], F32, kind="ExternalInput")
    convb_d = nc.dram_tensor("convb", [DIL, 1], F32, kind="ExternalInput")
    bdt_d = nc.dram_tensor("bdt", [DIL, 1], F32, kind="ExternalInput")
    identb_d = nc.dram_tensor("identb", [128, 128], BF16, kind="ExternalInput")
    diagd_d = nc.dram_tensor("diagd", [DIL, 128], BF16, kind="ExternalInput")
    # conv taps as diagonal matrices: [DC, NBLK, 128, 128]
    diagk_d = nc.dram_tensor("diagk", [DC, DIL, 128], BF16, kind="ExternalInput")
    out_d = nc.dram_tensor("out_p", [B, L, DM], F32, kind="ExternalOutput")

    ncha = L // LTA

    with tile.TileContext(nc) as tc:
        with (
            tc.tile_pool(name="weights", bufs=1) as wp,
            tc.tile_pool(name="resident", bufs=1) as rp,
            tc.tile_pool(name="dram", bufs=1, space="DRAM") as dp,
        ):
            # ---- weights to SBUF ----
            win_sb = wp.tile([128, KBLK, 2 * DIL], BF16)
            nc.sync.dma_start(win_sb[:], win_d[:].rearrange("(k p) m -> p k m", p=128))
            wout_sb = wp.tile([128, NBLK, DM], BF16)
            nc.sync.dma_start(wout_sb[:], wout_d[:].rearrange("(k p) m -> p k m", p=128))
            wx_sb = wp.tile([128, NBLK, 2 * DS], BF16)
            nc.sync.dma_start(wx_sb[:], wx_d[:].rearrange("(k p) m -> p k m", p=128))
            wdt_sb = wp.tile([DS, DIL], BF16)
            nc.sync.dma_start(wdt_sb[:], wdt_d[:])
            a_sb = wp.tile([128, NBLK, DS], F32)
            nc.sync.dma_start(a_sb[:], a_d[:].rearrange("(k p) m -> p k m", p=128))
            convb_sb = wp.tile([128, NBLK, 1], F32)
            nc.sync.dma_start(convb_sb[:], convb_d[:].rearrange("(k p) m -> p k m", p=128))
            bdt_sb = wp.tile([128, NBLK, 1], F32)
            nc.sync.dma_start(bdt_sb[:], bdt_d[:].rearrange("(k p) m -> p k m", p=128))
            identb_sb = wp.tile([128, 128], BF16)
            nc.sync.dma_start(identb_sb[:], identb_d[:])
            diagd_sb = wp.tile([128, NBLK, 128], BF16)
            nc.sync.dma_start(diagd_sb[:], diagd_d[:].rearrange("(k p) m -> p k m", p=128))
            diagk_sb = wp.tile([128, DC, NBLK, 128], BF16)
            nc.sync.dma_start(
                diagk_sb[:],
                diagk_d[:].rearrange("c (k p) m -> p c k m", p=128))

            # ---- resident activations (both batches) ----
            xcv = [[rp.tile([128, L], BF16, name=f"xcv{b_}{k}", tag=f"xcv{b_}{k}")
                    for k in range(NBLK)] for b_ in range(B)]
            zac = [[rp.tile([128, L], BF16, name=f"zac{b_}{k}", tag=f"zac{b_}{k}")
                    for k in range(NBLK)] for b_ in range(B)]
            md = [[rp.tile([128, L], BF16, name=f"md{b_}{k}", tag=f"md{b_}{k}")
                   for k in range(NBLK)] for b_ in range(B)]
            dtin_sb = [rp.tile([DS, L], BF16, name=f"dtin{b_}", tag=f"dtin{b_}")
                       for b_ in range(B)]
            # conv input with left halo, bf16, one per (b, blk)
            xp = [[rp.tile([128, L + DC - 1], BF16, name=f"xp{b_}{k}",
                           tag=f"xp{b_}{k}") for k in range(NBLK)]
                  for b_ in range(B)]
            yin = [[rp.tile([128, L], BF16, name=f"yin{b_}{k}", tag=f"yin{b_}{k}")
                    for k in range(NBLK)] for b_ in range(B)]

            cc_in = [dp.tile([2 * DS, L], BF16, name=f"cc_in{b_}") for b_ in range(B)]
            cc_out = [dp.tile([2 * DS, L], BF16, addr_space="Shared",
                              name=f"cc_out{b_}") for b_ in range(B)]

            # PSUM budget (8 banks of [128,512]f32): ps_in(2) + cps(1) +
            # ps_xs(1) + y_ps(4) = 8. ps_dt and ps_o reuse the ps_in tag.
            pools_cm = (
                tc.tile_pool(name="pa", bufs=2),
                tc.tile_pool(name="pa_ps", bufs=2, space="PSUM"),
                tc.tile_pool(name="pb", bufs=2),
                tc.tile_pool(name="pb_ps", bufs=1, space="PSUM"),
            )
            pa = pools_cm[0].__enter__()
            paps = pools_cm[1].__enter__()
            pb = pools_cm[2].__enter__()
            pbps = pools_cm[3].__enter__()

            def phase_a_chunk(b_, ch):
                """in_proj + conv(PE) + silu for a 512-token chunk; returns
                after feeding the x_proj partial into ps_xs."""
                t0 = ch * LTA
                xs_all = pa.tile([128, KBLK, LTA], BF16, tag="xs_all", bufs=3)
                nc.sync.dma_start(
                    xs_all[:],
                    x_t[b_].transpose([1, 0, 2])[:, :, t0:t0 + LTA])
                for m in range(2 * NBLK):
                    ps = paps.tile([128, LTA], F32, tag="ps_in", bufs=2)
                    for kb in range(KBLK):
                        nc.tensor.matmul(
                            ps[:],
                            win_sb[:, kb, m * 128:(m + 1) * 128],
                            xs_all[:, kb, :],
                            start=(kb == 0), stop=(kb == KBLK - 1))
                    if m < NBLK:  # x branch: conv via PE diag matmuls + silu
                        blk = m
                        if ch == 0:
                            nc.vector.memset(xp[b_][blk][:, 0:DC - 1], 0.0)
                        nc.scalar.copy(
                            xp[b_][blk][:, DC - 1 + t0:DC - 1 + t0 + LTA], ps[:])
                        cps = paps.tile([128, LTA], F32, tag="cps", bufs=1)
                        for k in range(DC):
                            nc.tensor.matmul(
                                cps[:],
                                diagk_sb[:, k, blk, :],
                                xp[b_][blk][:, t0 + k:t0 + k + LTA],
                                start=(k == 0), stop=(k == DC - 1))
                        nc.scalar.activation(
                            xcv[b_][blk][:, t0:t0 + LTA], cps[:],
                            AF.Silu, bias=convb_sb[:, blk, :])
                    else:  # z branch: silu
                        blk = m - NBLK
                        nc.scalar.activation(
                            zac[b_][blk][:, t0:t0 + LTA], ps[:], AF.Silu)
                # x_proj partial for this chunk
                ps_xs = paps.tile([2 * DS, LTA], F32, tag="ps_xs", bufs=1)
                for kb in range(NBLK):
                    nc.tensor.matmul(
                        ps_xs[:],
                        wx_sb[:, kb, :],
                        xcv[b_][kb][:, t0:t0 + LTA],
                        start=(kb == 0), stop=(kb == NBLK - 1))
                xs_sb = pa.tile([2 * DS, LTA], BF16, tag="xs_sb", bufs=2)
                nc.scalar.copy(xs_sb[:], ps_xs[:])
                nc.sync.dma_start(cc_in[b_][:, ch * LTA:(ch + 1) * LTA], xs_sb[:])

            def all_reduce(b_):
                nc.gpsimd.collective_compute(
                    "AllReduce", OP.add,
                    ins=[cc_in[b_].opt()], outs=[cc_out[b_].opt()],
                    replica_groups=[list(range(NCORES))])
                nc.sync.dma_start(dtin_sb[b_][:], cc_out[b_][0:DS, :])

            def dt_phase(b_):
                # md = -softplus(dt_raw + b_dt) = ln(sigmoid(-(dt_raw + b_dt)))
                LTD = 512
                for blk in range(NBLK):
                    for ch in range(L // LTD):
                        t0 = ch * LTD
                        ps_dt = paps.tile([128, LTD], F32, tag="ps_in", bufs=2)
                        nc.tensor.matmul(
                            ps_dt[:], wdt_sb[:, blk * 128:(blk + 1) * 128],
                            dtin_sb[b_][:, t0:t0 + LTD],
                            start=True, stop=True)
                        nc.scalar.activation(
                            md[b_][blk][:, t0:t0 + LTD], ps_dt[:],
                            AF.Sigmoid, bias=bdt_sb[:, blk, :], scale=-1.0)
                for blk in range(NBLK):
                    nc.scalar.activation(md[b_][blk][:], md[b_][blk][:], AF.Ln)

            def phase_b_prologue(b_, blk, dtx_on_dve=False):
                """dtx + D*x_conv accumulation start; returns y_ps tiles."""
                dtx = pb.tile([128, L], BF16, tag="dtx", bufs=2,
                              name=f"dtx{b_}{blk}")
                eng = nc.vector if dtx_on_dve else nc.gpsimd
                eng.tensor_mul(dtx[:], md[b_][blk][:], xcv[b_][blk][:])
                y_ps = [pbps.tile([128, 512], F32, tag=f"y_ps{pt}", bufs=1,
                                  name=f"yps{b_}{blk}{pt}")
                        for pt in range(NPT)]
                for pt in range(NPT):
                    nc.tensor.matmul(
                        y_ps[pt][:], diagd_sb[:, blk, :],
                        xcv[b_][blk][:, pt * 512:(pt + 1) * 512],
                        start=True, stop=False)
                return dtx, y_ps

            def phase_b_n(b_, blk, n, dtx, y_ps):
                bb = pb.tile([128, L], BF16, tag="bbn", bufs=3,
                             name=f"bb{b_}{blk}{n}")
                nc.sync.dma_start(
                    bb[:],
                    cc_out[b_][DS + n:DS + n + 1, :].broadcast_to([128, L]))
                da = pb.tile([128, L], F32, tag="dan", bufs=2,
                             name=f"da{b_}{blk}{n}")
                nc.scalar.activation(da[:], md[b_][blk][:], AF.Exp,
                                     scale=a_sb[:, blk, n:n + 1])
                u = pb.tile([128, L], BF16, tag="un", bufs=3,
                            name=f"u{b_}{blk}{n}")
                nc.vector.tensor_mul(u[:], dtx[:], bb[:])
                h = pb.tile([128, L], BF16, tag="hn", bufs=2,
                            name=f"h{b_}{blk}{n}")
                nc.vector.tensor_tensor_scan(h[:], da[:], u[:],
                                             0.0, OP.mult, OP.add)
                for pt in range(NPT):
                    nc.tensor.matmul(
                        y_ps[pt][:], identb_sb[:],
                        h[:, pt * 512:(pt + 1) * 512],
                        start=False, stop=(n == DS - 1))

            def phase_b_gate(b_, blk, y_ps):
                # gate from PSUM (DVE; pool cannot access PSUM)
                for pt in range(NPT):
                    nc.vector.tensor_mul(
                        yin[b_][blk][:, pt * 512:(pt + 1) * 512], y_ps[pt][:],
                        zac[b_][blk][:, pt * 512:(pt + 1) * 512])

            def phase_b_blk(b_, blk, dtx_on_dve=False):
                dtx, y_ps = phase_b_prologue(b_, blk, dtx_on_dve)
                for n in range(DS):
                    phase_b_n(b_, blk, n, dtx, y_ps)
                phase_b_gate(b_, blk, y_ps)

            def out_proj(b_):
                for mt in range(L // 128):
                    for dmh in range(2):
                        ps_o = paps.tile([128, 512], F32, tag="ps_in", bufs=2)
                        for blk in range(NBLK):
                            nc.tensor.matmul(
                                ps_o[:],
                                yin[b_][blk][:, mt * 128:(mt + 1) * 128],
                                wout_sb[:, blk, dmh * 512:(dmh + 1) * 512],
                                start=(blk == 0), stop=(blk == NBLK - 1))
                        osb = pb.tile([128, 512], F32, tag="osb", bufs=2)
                        nc.scalar.copy(osb[:], ps_o[:])
                        nc.sync.dma_start(
                            out_d[b_, mt * 128:(mt + 1) * 128,
                                  dmh * 512:(dmh + 1) * 512],
                            osb[:])

            # ---------------- emission schedule ----------------
            # A(b0) -> AR(b0) -> dt(b0) -> [A(b1) chunks interleaved with
            # B(b0,blk0) n-groups] -> AR(b1) -> dt(b1) -> B(b0,blk1) ->
            # out(b0) -> B(b1,*) -> out(b1)
            for ch in range(ncha):
                phase_a_chunk(0, ch)
            all_reduce(0)
            dt_phase(0)
            dtx0, y_ps0 = phase_b_prologue(0, 0, dtx_on_dve=True)
            for ch in range(ncha):
                phase_a_chunk(1, ch)
                for n in range(4 * ch, 4 * ch + 4):
                    phase_b_n(0, 0, n, dtx0, y_ps0)
            all_reduce(1)
            dt_phase(1)
            phase_b_gate(0, 0, y_ps0)
            phase_b_blk(0, 1)
            out_proj(0)
            phase_b_blk(1, 0)
            phase_b_blk(1, 1)
            out_proj(1)

            for cm in reversed(pools_cm):
                cm.__exit__(None, None, None)

    nc.compile()
    return nc


_NC_CACHE = {}


def _get_nc():
    if "nc" not in _NC_CACHE:
        _NC_CACHE["nc"] = build_nc()
    return _NC_CACHE["nc"]


def make_in_maps(x, W_in, conv_w, conv_b, W_x, W_dt, b_dt, A_log, D, W_out):
    x = np.asarray(x, np.float32)
    W_in = np.asarray(W_in, np.float32)
    conv_w = np.asarray(conv_w, np.float32)
    conv_b = np.asarray(conv_b, np.float32)
    W_x = np.asarray(W_x, np.float32)
    W_dt = np.asarray(W_dt, np.float32)
    b_dt = np.asarray(b_dt, np.float32)
    A_log = np.asarray(A_log, np.float32)
    D = np.asarray(D, np.float32)
    W_out = np.asarray(W_out, np.float32)

    xt = np.ascontiguousarray(x.transpose(0, 2, 1)).reshape(B, KBLK, 128, L).astype(ml_dtypes.bfloat16)
    A = np.exp(A_log)  # positive |A|; md = -dt on device

    in_maps = []
    for c in range(NCORES):
        lo = c * DIL
        sl = slice(lo, lo + DIL)
        cw = conv_w[sl]  # [DIL, DC]
        diagk = np.zeros((DC, DIL, 128), np.float32)
        for k in range(DC):
            for blk in range(NBLK):
                diagk[k, blk * 128:(blk + 1) * 128, :] = np.diag(
                    cw[blk * 128:(blk + 1) * 128, k])
        in_maps.append({
            "x_t": xt,
            "win": np.ascontiguousarray(
                np.concatenate([W_in[:, sl], W_in[:, DI + lo:DI + lo + DIL]],
                               axis=1)).astype(ml_dtypes.bfloat16),
            "wout": np.ascontiguousarray(W_out[sl]).astype(ml_dtypes.bfloat16),
            "wx": np.ascontiguousarray(
                np.concatenate([W_x[sl, :DS], -W_x[sl, DS:]], axis=1)
            ).astype(ml_dtypes.bfloat16),
            "wdt": np.ascontiguousarray(W_dt[:, sl]).astype(ml_dtypes.bfloat16),
            "a": np.ascontiguousarray(A[sl]),
            "convb": np.ascontiguousarray(conv_b[sl, None]),
            "bdt": np.ascontiguousarray(-b_dt[sl, None]),
            "identb": np.eye(128, dtype=ml_dtypes.bfloat16),
            "diagd": np.stack([np.diag(D[lo + k * 128:lo + (k + 1) * 128])
                               for k in range(NBLK)]).reshape(DIL, 128)
                       .astype(ml_dtypes.bfloat16),
            "diagk": diagk.astype(ml_dtypes.bfloat16),
        })
    return in_maps


def kernel(**inputs):
    nc = _get_nc()
    in_maps = make_in_maps(**inputs)
    res = run_bass_kernel_spmd(nc, in_maps, list(range(NCORES)))
    out = np.zeros((B, L, DM), np.float32)
    for c in range(NCORES):
        out += res.results[c]["out_p"]
    return out
